# revision 15
# baseline (speedup 1.0000x reference)
"""Trainium2 Bass kernel for the Kalman graphical-model message-passing problem.

reference math (B=64, D=8, M=4, S=50000):
    m1 = -Qinv @ (xs - F @ x_past)            (B, D, S)
    m2 = FtQinv @ (x_fut - F @ xs)            (B, D, S)
    m3 = HtRinv @ ys_t - (HtRinv @ H) @ xs    (B, D, S)
with x_past/x_fut edge-replicated 1-sample shifts of xs along S.

Reformulated as pure (tiny matrix) x (data) products with host-precomputed
weights:
    m1 = A1 @ xs + B1 @ x_past        A1 = -Qinv,        B1 = Qinv @ F
    m2 = A2 @ xs + B2 @ x_fut         A2 = -F'QinvF,     B2 = F' @ Qinv
    m3 = A3 @ xs + C3 @ ys_t          A3 = -(C3 @ H),    C3 = H' @ Rinv

All device I/O is bf16 (the 2e-2 rel-err budget dwarfs bf16's ~0.4%
rounding); the host converts inputs down and the outputs back up.  ys is
additionally transposed on host to (B, M, S) so the device consumes both
xs and ys in [small-dim, samples] rows — pure layout work, no host FLOPs.

Device layout (per core: bc=8 batches, data-parallel across 8 cores):
  Each batch's sample axis is split into npb = 16/bc equal streams of
  hs = S/npb samples; the 128 partitions hold all 16 (stream, batch) pairs
  x 8 states: partition 8*(p*bc + b) + j.  A supertile advances all 16
  streams together by TCG=2048 samples: SBUF X tile [128, 2050] with 1 halo
  column each side (cur/past/fut = column offsets 1/0/2), loaded by a single
  4-dim-AP DMA; only the first/last supertile need tiny halo fixups (stream
  boundaries inside a batch come free since streams are contiguous in DRAM).
  Y tile [64, 2048]: partition 4*(p*bc+b) + m.  Weights are block-diagonal
  lhsT matrices -> each output tile is a PSUM-accumulated chain of bf16
  matmuls in 512-column halves (PSUM bank limit):
    p0 = A1@cur + B1@past, p1 = A2@cur + B2@fut, p2 = A3@cur + Wy@y
  i.e. 6 matmul streams per half.  PSUM->SBUF bf16 drains alternate between
  the Vector and Scalar engines.  The three outputs live in one
  [bc, D, 3, s] DRAM tensor so each supertile's store is a single DMA whose
  (state, output) dims merge into one stride-s run of 24.
"""

import os
from contextlib import ExitStack

import ml_dtypes
import numpy as np

import concourse.bacc as bacc
import concourse.bass as bass
import concourse.mybir as mybir
import concourse.tile as tile
from concourse.bass_utils import run_bass_kernel_spmd

F32 = mybir.dt.float32
BF16 = mybir.dt.bfloat16
NP_BF16 = ml_dtypes.bfloat16

B, D, M, S = 64, 8, 4, 50000
N_CORES = 8
BC = B // N_CORES  # batches per core
NG = 16            # (stream, batch) groups packed into the 128 partitions
TCG = 4096         # samples per group per supertile
MW = 512           # matmul free-dim / PSUM bank width
NW = 6             # weight matrices in w_all


def _build_nc(bc=BC, s=S):
    variant = os.environ.get("KERNEL_VARIANT", "full")  # perf bisection only
    npb = NG // bc          # per-batch streams
    assert NG % bc == 0 and s % npb == 0, (bc, s)
    hs = s // npb           # samples per stream
    n_full = hs // TCG
    tc_tail = hs - n_full * TCG
    if tc_tail == 0:        # last full tile doubles as the tail
        n_full -= 1
        tc_tail = TCG
    n_tiles = n_full + 1
    pb = 8 * bc             # partitions per stream-block

    nc = bacc.Bacc(trn_type="TRN2")
    xs = nc.dram_tensor("xs", [bc, D, s], BF16, kind="ExternalInput")
    ys = nc.dram_tensor("ys", [bc, M, s], BF16, kind="ExternalInput")
    w = nc.dram_tensor("w_all", [128, NW * 128], BF16, kind="ExternalInput")
    # [b, j, o, s] layout: the store's (j, o) dims merge into one stride-s
    # run of 24, keeping the DMA access pattern at 3 dims + stream dim.
    m_all = nc.dram_tensor("m_all", [bc, D, 3, s], BF16, kind="ExternalOutput")

    with tile.TileContext(nc) as tc, ExitStack() as ctx:
        singles = ctx.enter_context(tc.tile_pool(name="singles", bufs=1))
        xp = ctx.enter_context(tc.tile_pool(name="xp", bufs=3))
        yp = ctx.enter_context(tc.tile_pool(name="yp", bufs=3))
        op = ctx.enter_context(tc.tile_pool(name="op", bufs=3))
        pp = ctx.enter_context(tc.tile_pool(name="pp", bufs=2, space="PSUM"))

        w_sb = singles.tile([128, NW * 128], BF16, tag="w")
        nc.sync.dma_start(out=w_sb[:], in_=w[:, :])
        wr = w_sb[:]

        for k in range(n_tiles):
            is_first = k == 0
            is_tail = k == n_tiles - 1
            tcw = tc_tail if is_tail else TCG
            base = k * TCG
            # tile columns c hold stream sample base-1+c
            c0 = 1 if is_first else 0
            c1 = tcw + 1 if is_tail else tcw + 2

            # --- load xs supertile (3-dim-AP DMA per stream + edge fixups):
            # 4-dim APs degrade to per-row descriptor generation on the
            # issuing queue (~55ns/row), so keep every DMA at <=3 dims.
            x_t = xp.tile([128, TCG + 2], BF16, tag="x")
            for p in range(npb):
                nc.sync.dma_start(
                    out=x_t[p * pb : (p + 1) * pb, c0:c1],
                    in_=bass.AP(
                        xs,
                        p * hs + base - 1 + c0,
                        [[D * s, bc], [s, D], [1, c1 - c0]],
                    ),
                )
            if is_first:
                # stream 0 of each batch: replicate sample 0 into the past
                # halo.  DVE needs a quadrant-aligned partition base: copy a
                # full aligned block; the halo DMA below then overwrites the
                # partitions belonging to later streams.
                q = 32 if pb <= 32 else pb
                nc.vector.tensor_copy(out=x_t[0:q, 0:1], in_=x_t[0:q, 1:2])
                if npb > 1:
                    # past halo for streams 1.. = previous stream's last
                    # sample (last dim is 1 wide, so this opts to 3 dims)
                    nc.sync.dma_start(
                        out=x_t[pb:128, 0:1],
                        in_=bass.AP(
                            xs,
                            hs - 1,
                            [[hs, npb - 1], [D * s, bc], [s, D], [1, 1]],
                        ),
                    )
            if is_tail:
                # last stream of each batch: replicate the final sample into
                # the fut halo (quadrant-aligned copy, fixed up below).
                q = 96 if pb <= 32 else 128 - pb
                nc.vector.tensor_copy(
                    out=x_t[q:128, tcw + 1 : tcw + 2],
                    in_=x_t[q:128, tcw : tcw + 1],
                )
                if npb > 1:
                    # fut halo for streams ..npb-2 = next stream's sample 0
                    nc.sync.dma_start(
                        out=x_t[0 : 128 - pb, tcw + 1 : tcw + 2],
                        in_=bass.AP(
                            xs,
                            hs,
                            [[hs, npb - 1], [D * s, bc], [s, D], [1, 1]],
                        ),
                    )

            # --- load ys supertile (partition 4g+m, columns=samples) -------
            y_t = yp.tile([64, TCG], BF16, tag="y")
            for p in range(npb):
                nc.gpsimd.dma_start(
                    out=y_t[p * 4 * bc : (p + 1) * 4 * bc, 0:tcw],
                    in_=bass.AP(
                        ys, p * hs + base, [[M * s, bc], [s, M], [1, tcw]]
                    ),
                )

            if variant == "loads":
                continue
            o_t = op.tile([128, 3 * TCG], BF16, tag="o", name=f"o_{k}")

            # --- matmuls + PSUM drain, in 512-column halves ----------------
            for hi, h0 in enumerate(range(0, tcw, MW)):
                hw_ = min(MW, tcw - h0)
                ps = [
                    pp.tile([128, MW], F32, tag=f"p{i}", name=f"p{i}_{k}_{h0}")
                    for i in range(3)
                ]
                cur = x_t[:, 1 + h0 : 1 + h0 + hw_]
                past = x_t[:, h0 : h0 + hw_]
                fut = x_t[:, 2 + h0 : 2 + h0 + hw_]
                p0 = ps[0][:, 0:hw_]
                p1 = ps[1][:, 0:hw_]
                p2 = ps[2][:, 0:hw_]
                nc.tensor.matmul(p0, wr[:, 0:128], cur, start=True, stop=False)
                nc.tensor.matmul(p0, wr[:, 128:256], past, start=False, stop=True)
                nc.tensor.matmul(p1, wr[:, 256:384], cur, start=True, stop=False)
                nc.tensor.matmul(p1, wr[:, 384:512], fut, start=False, stop=True)
                nc.tensor.matmul(p2, wr[:, 512:640], cur, start=True, stop=False)
                nc.tensor.matmul(
                    p2,
                    wr[0:64, 640:768],
                    y_t[0:64, h0 : h0 + hw_],
                    start=False,
                    stop=True,
                )
                if variant == "nostores":
                    continue
                # drain PSUM -> bf16 o_t, alternating Vector / Scalar
                # (GPSIMD cannot read PSUM)
                for i in range(3):
                    out_ap = o_t[:, i * tcw + h0 : i * tcw + h0 + hw_]
                    if (hi + i) % 2 == 0:
                        nc.vector.tensor_copy(out=out_ap, in_=ps[i][:, 0:hw_])
                    else:
                        nc.scalar.copy(out=out_ap, in_=ps[i][:, 0:hw_])

            if variant == "nostores":
                continue
            # --- one merged store DMA per supertile and stream-block -------
            o_r = o_t[:, 0 : 3 * tcw].rearrange("p (o t) -> p o t", o=3)
            for p in range(npb):
                nc.scalar.dma_start(
                    out=bass.AP(
                        m_all,
                        p * hs + base,
                        [[3 * D * s, bc], [s, 3 * D], [1, tcw]],
                    ),
                    in_=o_r[p * pb : (p + 1) * pb],
                )
    nc.finalize()
    return nc


def _build_weights(F, H, Q, R):
    """Host-side precompute (init-time work in the torch module)."""
    F64 = np.asarray(F, np.float64)
    H64 = np.asarray(H, np.float64)
    Q64 = np.asarray(Q, np.float64)
    R64 = np.asarray(R, np.float64)
    Qinv = np.linalg.inv(Q64)
    Rinv = np.linalg.inv(R64)
    A1 = -Qinv
    B1 = Qinv @ F64
    B2 = F64.T @ Qinv
    A2 = -(B2 @ F64)
    C3 = H64.T @ Rinv          # (D, M)
    A3 = -(C3 @ H64)

    w = np.zeros((128, NW * 128), np.float32)
    eye = np.eye(NG)
    for i, A in enumerate([A1, B1, A2, B2, A3]):
        # lhsT[8g+j, 8g+i] = A[i, j]  ->  block-diag of A.T
        w[:, i * 128 : (i + 1) * 128] = np.kron(eye, A.T).astype(np.float32)
    # Wy: lhsT[4g+m, 8g+i] = C3[i, m]
    for g in range(NG):
        for m in range(M):
            w[4 * g + m, 5 * 128 + 8 * g : 5 * 128 + 8 * g + D] = C3[:, m]
    return w


_CACHE = {}


def _get_nc(bc=BC, s=S):
    key = (bc, s)
    if key not in _CACHE:
        _CACHE[key] = _build_nc(bc, s)
    return _CACHE[key]


def run(xs, ys, F, H, Q, R, trace=False, bc=BC, s=S):
    """Shard across 8 cores, run, gather.  Returns ((m1, m2, m3), results)."""
    xs = np.ascontiguousarray(np.asarray(xs, np.float32).astype(NP_BF16))
    # transpose ys to (B, M, S) rows on host (layout only)
    ys = np.asarray(ys, np.float32).swapaxes(1, 2).astype(NP_BF16, order="C")
    w_all = _build_weights(F, H, Q, R).astype(NP_BF16)
    nb = xs.shape[0]
    assert nb == bc * N_CORES and xs.shape[1:] == (D, s), xs.shape
    assert ys.shape == (nb, M, s), ys.shape

    nc = _get_nc(bc, s)
    in_maps = [
        {
            "xs": np.ascontiguousarray(xs[i * bc : (i + 1) * bc]),
            "ys": np.ascontiguousarray(ys[i * bc : (i + 1) * bc]),
            "w_all": w_all,
        }
        for i in range(N_CORES)
    ]
    res = run_bass_kernel_spmd(nc, in_maps, core_ids=list(range(N_CORES)), trace=trace)
    m_full = np.concatenate([r["m_all"] for r in res.results], axis=0)  # (B,D,3,s)
    outs = tuple(
        np.ascontiguousarray(m_full[:, :, i, :]).astype(np.float32) for i in range(3)
    )
    return outs, res


def kernel(xs, ys, F, H, Q, R):
    trace = bool(int(os.environ.get("KERNEL_TRACE", "0")))
    outs, _ = run(xs, ys, F, H, Q, R, trace=trace)
    return outs
